# revision 25
# baseline (speedup 1.0000x reference)
"""Additive-attention kernel for Trainium2 (8 NeuronCores, SPMD).

Problem (per batch b of B=4):
    xt      = x[b].T                                  # (N=512, D=96)
    g1      = xt @ Wg1.T                              # (512, 256)
    g2      = xt @ Wg2.T                              # (512, 256)
    score   = sum_a Wa[a] * tanh(g1[n,a] + g2[m,a] + bg[a])    # (512, 512)
    att     = sigmoid(score + Wa_b + ba)
    out[b]  = att @ xt                                # (512, 96)

Sharding: core c handles batch b = c//2 and query-rows n in
[(c%2)*256, (c%2)*256+256).  Keys are PERMUTED per core (own-half keys
first) so the query slice xq is a fixed-offset view of the key tile;
the sum over keys is permutation-invariant.

Algorithm (v7): tanh(u+v) ~= sum_{j in 1,3,5} BJ_j * sin(j*S*(u+v)),
S = pi/8.4 (weighted LSQ fit of tanh on |t|<=9, w=N(0,1.4^2)+0.05).
Each harmonic separates, so the N x N score becomes matmuls over a
contraction of (a, j, sin|cos) = 1536.  ACT's sin spline only covers
|x| < 4 (profile exponent buckets), so ONLY the j=1 seeds may use ACT
Sin; j=3,5 come from the f16 DVE recurrence
    f3 = f1*(2cos2t +- 1),  f5 = f3*2cos2t - f1.

Scheduling notes (hard-won; ~15% run-to-run device-state variance,
so compare min-of-3):
- Engines only start after the preamble barrier (~7.0us); a HWDGE
  dma_start costs ~0.6-0.7us of descriptor-gen ON the issuing engine
  and first bytes land ~0.6us later.  The thu inputs (w1|xq) get the
  sync ring alone; the rest rides gpsimd SWDGE.  NEVER dma_start on
  the scalar ring: it blocks the ACT queue ~1.3us AND makes the
  act-table pass emit an extra exp_and_others load.
- A single silu-dummy activation forces ONE load of silu_and_others
  (the only ACT table set with sin AND tanh).  ACT's sin spline only
  covers |x| < 4, so Sin(scale=5*theta) silently breaks.
- Tile deps are TILE-granular and EMISSION-ORDER based (readers must
  be emitted after their writers); the Tile scheduler may reorder
  same-engine instructions subject to deps, so "filler" matmuls must
  be anchored on data that arrives when you want them to run.
- PE warms up (HAM 4/8 -> 8/8 needs ~3.4us of sustained activity) on
  ones-matmuls into the thv PSUM region + anchored fillers into
  score banks, so score MMs run at 2.4GHz (109ns vs 213ns each).
- j1/j3 score-MM timing has slack (PE ~45% busy); only the j5-stop
  -> sigmoid -> out chain is critical.  Hence us3/us5 feature
  scaling runs on ACT (Identity, per-partition scale column) in its
  post-seed idle window, the u5 chain sits early in the DVE spine,
  and the v5 chain (tv=cv3*2cos2t, cv5=tv-cv1) is the spine tail,
  split per key-half with cos lanes first so fn=0 stop MMs start
  ASAP.
- sigmoid rewritten 0.5+0.5*tanh(0.5*s): the 0.5 offset becomes a
  ones-colsum matmul and both 0.5 factors are pre-folded into xkTP
  on the host.  The output accumulates TRANSPOSED (foT[d,n], with
  xkT stationary) so attT streams as matmul rhs with no per-half
  re-LDW; the host transposes back.
"""

import numpy as np

B, D, N, A = 4, 96, 512, 256
NH = N // 2          # query rows per core
NCORES = 8

JS = (1, 3, 5)
FL = 8.4
FS = float(np.pi / FL)
BJ = {1: 1.206938, 3: 0.263773, 5: 0.089706}

_cache = {}


def _build_nc_v7(bg_zero=False):
    import concourse.bacc as bacc
    import concourse.mybir as mybir
    from concourse import tile

    f32 = mybir.dt.float32
    f16 = mybir.dt.float16
    AF = mybir.ActivationFunctionType
    MULT = mybir.AluOpType.mult
    ADD = mybir.AluOpType.add

    nc = bacc.Bacc("TRN2", target_bir_lowering=False)

    # xkb = xkTP [128, 4*D] ++ f16 bias columns:
    #   wav(2) sgb(1) wab3(2) wab5(2) [+ bg: b1s(2) b1c(2)]
    NBC = 7 if bg_zero else 11
    vin = nc.dram_tensor("vin", [D, 2 * A + N], f16, kind="ExternalInput")
    xkb = nc.dram_tensor("xkb", [128, 4 * D + NBC], f16,
                         kind="ExternalInput")
    out = nc.dram_tensor("out", [D, NH], f16, kind="ExternalOutput")

    FU = NH * 2          # 512: u-side feature width (2 a-chunks)

    with tile.TileContext(nc) as tc:
        with (
            tc.tile_pool(name="consts", bufs=1) as consts,
            tc.tile_pool(name="feat", bufs=1) as feat,
            tc.tile_pool(name="gps", bufs=1, space="PSUM") as gps,
            tc.tile_pool(name="scps", bufs=1, space="PSUM") as scps,
            tc.tile_pool(name="opool", bufs=1) as opool,
        ):
            vin_sb = consts.tile([D, 2 * A + N], f16, tag="vin")
            xkb_sb = consts.tile([128, 4 * D + NBC], f16, tag="xkb")
            # vin layout: [w1 | xk | w2]; xq = first NH key columns
            w1_sb = vin_sb[:, :A]
            xk_sb = vin_sb[:, A:A + N]
            xq_sb = xk_sb[:, :NH]
            w2_sb = vin_sb[:, A + N:]
            xkT_sb = xkb_sb[:, :4 * D].rearrange("p (mb d) -> p mb d", d=D)
            # bias columns ride as f16; widen to f32 once on-device
            # (tensor_scalar scalar1 and ACT bias APs must be f32)
            biasf = consts.tile([128, NBC], f32, tag="biasf")
            wav_sb = biasf[:, 0:2]
            sgb_sb = biasf[:, 2:3]
            wab3_sb = biasf[:, 3:5]
            wab5_sb = biasf[:, 5:7]
            if not bg_zero:
                b1s_sb = biasf[:, 7:9]
                b1c_sb = biasf[:, 9:11]

            # gpsimd owns the const memsets (it is otherwise idle and
            # starts right after the barrier, freeing DVE's queue)
            ones = consts.tile([128, 512], f16, tag="ones")
            nc.gpsimd.memset(ones[:], 1.0)
            hpi = consts.tile([128, 1], f32, tag="hpi")
            nc.gpsimd.memset(hpi[:], float(np.pi / 2))
            dsil = consts.tile([128, 1], f32, tag="dsil")
            nc.gpsimd.memset(dsil[:], 0.0)

            # silu-dummy: forces ONE load of silu_and_others, then the
            # xkb DMA rides the scalar HWDGE ring (vin owns sync's)
            nc.scalar.activation(dsil[:], dsil[:], AF.Silu)
            nc.sync.dma_start(vin_sb[:, :A + NH], vin.ap()[:, :A + NH])
            nc.gpsimd.dma_start(vin_sb[:, A + NH:], vin.ap()[:, A + NH:])
            nc.gpsimd.dma_start(xkb_sb[:], xkb.ap())

            thu = gps.tile([128, FU], f32, tag="thu", name="thu")
            thv = [gps.tile([128, N], f32, tag=f"thv{c}", name=f"thv{c}")
                   for c in range(2)]
            # PE warmup into thv0's region (overwritten by the real MM)
            for _ in range(4):
                nc.tensor.matmul(thv[0][:], ones[:, :128], ones[:])
            for c in range(2):
                nc.tensor.matmul(thu[:, c * NH:(c + 1) * NH],
                                 w1_sb[:, c * 128:(c + 1) * 128], xq_sb[:])
            for c in range(2):
                nc.tensor.matmul(thv[c][:],
                                 w2_sb[:, c * 128:(c + 1) * 128], xk_sb[:])

            # constant half-sum term, transposed: foT[d, n] starts as
            # sum_m xkTP[m, d] (xkT is the stationary side so the later
            # att matmuls stream attT with free-dim NH and no re-LDW
            # per n-half)
            foT = gps.tile([D, NH], f32, tag="foT", name="foT")
            for mb in range(4):
                nc.tensor.matmul(
                    foT[:], xkT_sb[:, mb, :], ones[:, :NH],
                    start=(mb == 0), stop=False, skip_group_check=True,
                )

            # per-lane feature tiles (separate tiles per sin|cos lane so
            # ACT lane writes never false-serialize against DVE readers)
            # u side: [128, FU]; v side: [128, 2(chunk), N]
            # u-side tiles carry both lanes [128, 2(sin|cos), FU] (all
            # DVE-written, so no cross-engine false deps); v-side seed
            # and j3 tiles are split per lane (ACT lane writes / early
            # cos-lane consumers), cv5 is one both-lane tile.
            cu = {j: feat.tile([128, 2, FU], f16, tag=f"cu{j}", name=f"cu{j}")
                  for j in JS}
            us = {j: feat.tile([128, 2, FU], f16, tag=f"us{j}", name=f"us{j}")
                  for j in (1,)}
            cvs = {j: feat.tile([128, 2, N], f16, tag=f"cvs{j}", name=f"cvs{j}") for j in (1, 3)}
            cvc = {j: feat.tile([128, 2, N], f16, tag=f"cvc{j}", name=f"cvc{j}") for j in (1, 3)}

            def useed(lane):
                if bg_zero:
                    bias = hpi[:] if lane else 0.0
                    nc.scalar.activation(cu[1][:, lane, :], thu[:], AF.Sin,
                                         bias=bias)
                else:
                    bl = b1c_sb if lane else b1s_sb
                    for c in range(2):
                        nc.scalar.activation(
                            cu[1][:, lane, c * NH:(c + 1) * NH],
                            thu[:, c * NH:(c + 1) * NH], AF.Sin,
                            bias=bl[:, c:c + 1])

            def vseed(lane, tile_, c):
                if bg_zero:
                    bias = hpi[:] if lane else 0.0
                else:
                    bias = (b1c_sb if lane else b1s_sb)[:, c:c + 1]
                nc.scalar.activation(tile_[:, c, :], thv[c][:], AF.Sin,
                                     bias=bias)

            useed(1)
            useed(0)
            vseed(1, cvc[1], 0)
            vseed(1, cvc[1], 1)
            # v-side square on ACT (Square is in the silu set): removes
            # 0.7us from the DVE spine; the delayed vsin only gates
            # slack-tolerant fn=1 consumers
            sqv = feat.tile([128, 2, N], f16, tag="sqv")
            nc.scalar.activation(sqv[:], cvc[1][:], AF.Square)
            vseed(0, cvs[1], 0)
            vseed(0, cvs[1], 1)


            def uscale(j):
                for c in range(2):
                    nc.vector.tensor_scalar(
                        us[j][:, :, c * NH:(c + 1) * NH],
                        cu[j][:, :, c * NH:(c + 1) * NH],
                        wav_sb[:, c:c + 1], float(BJ[j]), MULT, MULT)

            sc = [scps.tile([128, NH], f32, tag=f"sc{mb}", name=f"sc{mb}")
                  for mb in range(4)]

            def score_mms(j, rhs_t, first=False):
                lhs = {0: cvc[j], 1: cvs[j]}  # fn=0 pairs us-sin x cv-cos
                for fn in range(2):
                    for c in range(2):
                        for mb in range(4):
                            nc.tensor.matmul(
                                sc[mb][:],
                                lhs[fn][:, c, mb * 128:(mb + 1) * 128],
                                rhs_t[:, fn, c * NH:(c + 1) * NH],
                                start=(first and fn == 0 and c == 0),
                                stop=False,
                                skip_group_check=True,
                            )

            # ---- DVE program (emission order = engine order) ----
            squ = feat.tile([128, FU], f16, tag="squ")
            t2u = feat.tile([128, FU], f16, tag="t2u")
            m3u = feat.tile([128, 2, FU], f16, tag="m3u")
            nc.gpsimd.tensor_copy(biasf[:], xkb_sb[:, 4 * D:])
            nc.vector.tensor_mul(squ[:], cu[1][:, 1, :], cu[1][:, 1, :])
            nc.vector.tensor_scalar(t2u[:], squ[:], 4.0, -2.0, MULT, ADD)
            uscale(1)
            nc.vector.tensor_scalar(m3u[:, 0, :], squ[:], 4.0, -1.0,
                                    MULT, ADD)
            nc.vector.tensor_scalar(m3u[:, 1, :], squ[:], 4.0, -3.0,
                                    MULT, ADD)
            nc.vector.tensor_mul(cu[3][:], cu[1][:], m3u[:])
            # u5 next so ACT can scale us3/us5 in its post-seed window
            t2u_b = t2u[:, None, :].broadcast_to((128, 2, FU))
            tu = feat.tile([128, 2, FU], f16, tag="tu")
            nc.vector.tensor_mul(tu[:], cu[3][:], t2u_b)
            nc.vector.tensor_sub(cu[5][:], tu[:], cu[1][:])
            us3t = feat.tile([128, 2, FU], f16, tag="us3t")
            us5t = feat.tile([128, 2, FU], f16, tag="us5t")
            for c in range(2):
                nc.scalar.activation(
                    us3t[:, :, c * NH:(c + 1) * NH],
                    cu[3][:, :, c * NH:(c + 1) * NH],
                    AF.Identity, scale=wab3_sb[:, c:c + 1])
            for c in range(2):
                nc.scalar.activation(
                    us5t[:, :, c * NH:(c + 1) * NH],
                    cu[5][:, :, c * NH:(c + 1) * NH],
                    AF.Identity, scale=wab5_sb[:, c:c + 1])
            m3vp = feat.tile([128, 2, N], f16, tag="m3vp")
            m3vm = feat.tile([128, 2, N], f16, tag="m3vm")
            t2v = feat.tile([128, 2, N], f16, tag="t2v")
            nc.vector.tensor_scalar(t2v[:], sqv[:], 4.0, -2.0, MULT, ADD)
            nc.vector.tensor_scalar(m3vm[:], sqv[:], 4.0, -3.0, MULT, ADD)
            nc.vector.tensor_mul(cvc[3][:], cvc[1][:], m3vm[:])
            nc.vector.tensor_scalar(m3vp[:], sqv[:], 4.0, -1.0, MULT, ADD)
            nc.vector.tensor_mul(cvs[3][:], cvs[1][:], m3vp[:])
            # ---- PE score program ----
            # filler ladder: each rung anchored on successively later
            # data so the scheduler cannot hoist them; keeps HAM warm
            # from first warmup MM through the j1 score MMs.  The j1
            # start=True MMs re-clear the banks afterwards.
            for f in range(4):
                nc.tensor.matmul(sc[1][:96, :], xkT_sb[:, f, :],
                                 ones[:, :NH], skip_group_check=True)
            for f in range(2):
                nc.tensor.matmul(sc[2][:], cu[1][:, 1, :128],
                                 ones[:, :NH], skip_group_check=True)
            for f in range(2):
                nc.tensor.matmul(sc[3][:], us[1][:, 0, :128],
                                 ones[:, :NH], skip_group_check=True)
            score_mms(1, us[1], first=True)
            score_mms(3, us3t)

            # j5 v-chain emitted per key-half, interleaved with the j5
            # score MMs + sigmoid + out MMs of the matching mb pair
            tv = feat.tile([128, 2, 2, N], f16, tag="tv")
            cv5 = feat.tile([128, 2, 2, N], f16, tag="cv5")
            attT = [consts.tile([128, NH], f16, tag=f"attT{mb}", name=f"attT{mb}")
                    for mb in range(4)]
            t2v_b = t2v[:, None, :, :].broadcast_to((128, 2, 2, N))
            cv1l = {0: cvs[1], 1: cvc[1]}
            cv3l = {0: cvs[3], 1: cvc[3]}
            for h in range(2):
                sl = slice(h * 256, (h + 1) * 256)
                for ln in (1, 0):
                    nc.vector.tensor_mul(
                        tv[:, ln, :, sl], cv3l[ln][:, :, sl],
                        t2v_b[:, ln, :, sl])
                    nc.vector.tensor_sub(
                        cv5[:, ln, :, sl], tv[:, ln, :, sl],
                        cv1l[ln][:, :, sl])
                for mb in (2 * h, 2 * h + 1):
                    for fn in range(2):
                        for c in range(2):
                            nc.tensor.matmul(
                                sc[mb][:],
                                cv5[:, 1 - fn, c, mb * 128:(mb + 1) * 128],
                                us5t[:, fn, c * NH:(c + 1) * NH],
                                start=False,
                                stop=(fn == 1 and c == 1),
                                skip_group_check=True,
                            )
                    nc.scalar.activation(
                        attT[mb][:], sc[mb][:], AF.Tanh, scale=0.5,
                        bias=sgb_sb[:, 0:1]
                    )
                    nc.tensor.matmul(
                        foT[:], xkT_sb[:, mb, :], attT[mb][:],
                        start=False, stop=(mb == 3),
                        skip_group_check=True,
                    )

            out_sb = opool.tile([D, NH], f16, tag="out")
            nc.vector.tensor_copy(out_sb[:, :NH // 2], foT[:, :NH // 2])
            nc.sync.dma_start(out.ap()[:, :NH // 2], out_sb[:, :NH // 2])
            nc.scalar.copy(out_sb[:, NH // 2:], foT[:, NH // 2:])
            nc.sync.dma_start(out.ap()[:, NH // 2:], out_sb[:, NH // 2:])

    nc.compile()
    return nc


def _prep_inputs_v7(x, Wg1, Wg2, bg, Wa_w, Wa_b, ba, bg_zero):
    """Host-side packing/slicing only (no reference math)."""
    x = np.asarray(x, np.float32)
    w1s = (FS * np.asarray(Wg1, np.float32).T).astype(np.float16)
    w2s = (FS * np.asarray(Wg2, np.float32).T).astype(np.float16)
    wac = np.asarray(Wa_w, np.float32).reshape(2, 128).T
    NBC = 7 if bg_zero else 11
    biasc = np.empty((128, NBC), np.float16)
    biasc[:, 0:2] = wac.astype(np.float16)
    biasc[:, 2] = np.float16(0.5 * (float(np.asarray(Wa_b).ravel()[0])
                                    + float(np.asarray(ba).ravel()[0])))
    biasc[:, 3:5] = (wac * np.float32(BJ[3])).astype(np.float16)
    biasc[:, 5:7] = (wac * np.float32(BJ[5])).astype(np.float16)
    if not bg_zero:
        bgv = FS * np.asarray(bg, np.float32)
        biasc[:, 7:9] = bgv.reshape(2, 128).T.astype(np.float16)
        biasc[:, 9:11] = (bgv.reshape(2, 128).T
                          + np.float32(np.pi / 2)).astype(np.float16)
    in_maps = []
    for c in range(NCORES):
        b, half = c // 2, c % 2
        xb = x[b]
        # per-core key permutation: own-half keys first, so xq is a
        # fixed-offset view of xk in every core's (identical) program
        xp = np.concatenate(
            [xb[:, half * NH:(half + 1) * NH],
             xb[:, (1 - half) * NH:(2 - half) * NH]], axis=1)
        vin = np.ascontiguousarray(
            np.concatenate([w1s, xp.astype(np.float16), w2s], axis=1))
        xkTP = ((0.5 * xp.T).astype(np.float16).reshape(4, 128, D)
                .transpose(1, 0, 2).reshape(128, 4 * D))
        xkb = np.ascontiguousarray(np.concatenate([xkTP, biasc], axis=1))
        in_maps.append({"vin": vin, "xkb": xkb})
    return in_maps


def _run(inputs, trace=False):
    from concourse.bass_utils import run_bass_kernel_spmd

    bg_zero = bool(np.all(np.asarray(inputs["bg"]) == 0))
    key = ("nc7b", bg_zero)
    if key not in _cache:
        _cache[key] = _build_nc_v7(bg_zero=bg_zero)
    nc = _cache[key]
    in_maps = _prep_inputs_v7(**inputs, bg_zero=bg_zero)
    res = run_bass_kernel_spmd(
        nc, in_maps, core_ids=list(range(NCORES)), trace=trace
    )
    out = np.empty((B, N, D), np.float32)
    for c in range(NCORES):
        b, half = c // 2, c % 2
        out[b, half * NH:(half + 1) * NH] = \
            res.results[c]["out"].astype(np.float32).T
    return out, res


def kernel(**inputs):
    out, _ = _run(inputs, trace=False)
    return out


# revision 26
# speedup vs baseline: 1.1558x; 1.1558x over previous
"""Additive-attention kernel for Trainium2 (8 NeuronCores, SPMD).

Problem (per batch b of B=4):
    xt      = x[b].T                                  # (N=512, D=96)
    g1      = xt @ Wg1.T                              # (512, 256)
    g2      = xt @ Wg2.T                              # (512, 256)
    score   = sum_a Wa[a] * tanh(g1[n,a] + g2[m,a] + bg[a])    # (512, 512)
    att     = sigmoid(score + Wa_b + ba)
    out[b]  = att @ xt                                # (512, 96)

Sharding: core c handles batch b = c//2 and query-rows n in
[(c%2)*256, (c%2)*256+256).  Keys are PERMUTED per core (own-half keys
first) so the query slice xq is a fixed-offset view of the key tile;
the sum over keys is permutation-invariant.

Algorithm (v7): tanh(u+v) ~= sum_{j in 1,3,5} BJ_j * sin(j*S*(u+v)),
S = pi/8.4 (weighted LSQ fit of tanh on |t|<=9, w=N(0,1.4^2)+0.05).
Each harmonic separates, so the N x N score becomes matmuls over a
contraction of (a, j, sin|cos) = 1536.  ACT's sin spline only covers
|x| < 4 (profile exponent buckets), so ONLY the j=1 seeds may use ACT
Sin; j=3,5 come from the f16 DVE recurrence
    f3 = f1*(2cos2t +- 1),  f5 = f3*2cos2t - f1.

Scheduling notes (hard-won; ~15% run-to-run device-state variance,
so compare min-of-3):
- Engines only start after the preamble barrier (~7.0us); a HWDGE
  dma_start costs ~0.6-0.7us of descriptor-gen ON the issuing engine
  and first bytes land ~0.6us later.  The thu inputs (w1|xq) get the
  sync ring alone; the rest rides gpsimd SWDGE.  NEVER dma_start on
  the scalar ring: it blocks the ACT queue ~1.3us AND makes the
  act-table pass emit an extra exp_and_others load.
- A single silu-dummy activation forces ONE load of silu_and_others
  (the only ACT table set with sin AND tanh).  ACT's sin spline only
  covers |x| < 4, so Sin(scale=5*theta) silently breaks.
- Tile deps are TILE-granular and EMISSION-ORDER based (readers must
  be emitted after their writers); the Tile scheduler may reorder
  same-engine instructions subject to deps, so "filler" matmuls must
  be anchored on data that arrives when you want them to run.
- PE warms up (HAM 4/8 -> 8/8 needs ~3.4us of sustained activity) on
  ones-matmuls into the thv PSUM region + anchored fillers into
  score banks, so score MMs run at 2.4GHz (109ns vs 213ns each).
- j1/j3 score-MM timing has slack (PE ~45% busy); only the j5-stop
  -> sigmoid -> out chain is critical.  Hence us3/us5 feature
  scaling runs on ACT (Identity, per-partition scale column) in its
  post-seed idle window, the u5 chain sits early in the DVE spine,
  and the v5 chain (tv=cv3*2cos2t, cv5=tv-cv1) is the spine tail,
  split per key-half with cos lanes first so fn=0 stop MMs start
  ASAP.
- sigmoid rewritten 0.5+0.5*tanh(0.5*s): the 0.5 offset becomes a
  ones-colsum matmul and both 0.5 factors are pre-folded into xkTP
  on the host.  The output accumulates TRANSPOSED (foT[d,n], with
  xkT stationary) so attT streams as matmul rhs with no per-half
  re-LDW; the host transposes back.
"""

import numpy as np

B, D, N, A = 4, 96, 512, 256
NH = N // 2          # query rows per core
NCORES = 8

JS = (1, 3, 5)
FL = 8.4
FS = float(np.pi / FL)
BJ = {1: 1.206938, 3: 0.263773, 5: 0.089706}

_cache = {}


def _build_nc_v7(bg_zero=False):
    import concourse.bacc as bacc
    import concourse.mybir as mybir
    from concourse import tile

    f32 = mybir.dt.float32
    f16 = mybir.dt.float16
    AF = mybir.ActivationFunctionType
    MULT = mybir.AluOpType.mult
    ADD = mybir.AluOpType.add

    nc = bacc.Bacc("TRN2", target_bir_lowering=False)

    # xkb = xkTP [128, 4*D] ++ f16 bias columns:
    #   wav(2) sgb(1) wab3(2) wab5(2) [+ bg: b1s(2) b1c(2)]
    NBC = 7 if bg_zero else 11
    vin = nc.dram_tensor("vin", [D, 2 * A + N], f16, kind="ExternalInput")
    xkb = nc.dram_tensor("xkb", [128, 4 * D + NBC], f16,
                         kind="ExternalInput")
    out = nc.dram_tensor("out", [D, NH], f16, kind="ExternalOutput")

    FU = NH * 2          # 512: u-side feature width (2 a-chunks)

    with tile.TileContext(nc) as tc:
        with (
            tc.tile_pool(name="consts", bufs=1) as consts,
            tc.tile_pool(name="feat", bufs=1) as feat,
            tc.tile_pool(name="gps", bufs=1, space="PSUM") as gps,
            tc.tile_pool(name="scps", bufs=1, space="PSUM") as scps,
            tc.tile_pool(name="opool", bufs=1) as opool,
        ):
            vin_sb = consts.tile([D, 2 * A + N], f16, tag="vin")
            xkb_sb = consts.tile([128, 4 * D + NBC], f16, tag="xkb")
            # vin layout: [w1 | xk | w2]; xq = first NH key columns
            w1_sb = vin_sb[:, :A]
            xk_sb = vin_sb[:, A:A + N]
            xq_sb = xk_sb[:, :NH]
            w2_sb = vin_sb[:, A + N:]
            xkT_sb = xkb_sb[:, :4 * D].rearrange("p (mb d) -> p mb d", d=D)
            # bias columns ride as f16; widen to f32 once on-device
            # (tensor_scalar scalar1 and ACT bias APs must be f32)
            biasf = consts.tile([128, NBC], f32, tag="biasf")
            wav_sb = biasf[:, 0:2]
            sgb_sb = biasf[:, 2:3]
            wab3_sb = biasf[:, 3:5]
            wab5_sb = biasf[:, 5:7]
            if not bg_zero:
                b1s_sb = biasf[:, 7:9]
                b1c_sb = biasf[:, 9:11]

            # gpsimd owns the const memsets (it is otherwise idle and
            # starts right after the barrier, freeing DVE's queue)
            ones = consts.tile([128, 512], f16, tag="ones")
            nc.gpsimd.memset(ones[:], 1.0)
            hpi = consts.tile([128, 1], f32, tag="hpi")
            nc.gpsimd.memset(hpi[:], float(np.pi / 2))
            dsil = consts.tile([128, 1], f32, tag="dsil")
            nc.gpsimd.memset(dsil[:], 0.0)

            # silu-dummy: forces ONE load of silu_and_others, then the
            # xkb DMA rides the scalar HWDGE ring (vin owns sync's)
            nc.scalar.activation(dsil[:], dsil[:], AF.Silu)
            nc.sync.dma_start(vin_sb[:, :A + NH], vin.ap()[:, :A + NH])
            nc.gpsimd.dma_start(vin_sb[:, A + NH:], vin.ap()[:, A + NH:])
            nc.gpsimd.dma_start(xkb_sb[:], xkb.ap())

            thu = gps.tile([128, FU], f32, tag="thu", name="thu")
            thv = [gps.tile([128, N], f32, tag=f"thv{c}", name=f"thv{c}")
                   for c in range(2)]
            # PE warmup into thv0's region (overwritten by the real MM)
            for _ in range(4):
                nc.tensor.matmul(thv[0][:], ones[:, :128], ones[:])
            for c in range(2):
                nc.tensor.matmul(thu[:, c * NH:(c + 1) * NH],
                                 w1_sb[:, c * 128:(c + 1) * 128], xq_sb[:])
            for c in range(2):
                nc.tensor.matmul(thv[c][:],
                                 w2_sb[:, c * 128:(c + 1) * 128], xk_sb[:])

            # constant half-sum term, transposed: foT[d, n] starts as
            # sum_m xkTP[m, d] (xkT is the stationary side so the later
            # att matmuls stream attT with free-dim NH and no re-LDW
            # per n-half)
            foT = gps.tile([D, NH], f32, tag="foT", name="foT")
            for mb in range(4):
                nc.tensor.matmul(
                    foT[:], xkT_sb[:, mb, :], ones[:, :NH],
                    start=(mb == 0), stop=False, skip_group_check=True,
                )

            # per-lane feature tiles (separate tiles per sin|cos lane so
            # ACT lane writes never false-serialize against DVE readers)
            # u side: [128, FU]; v side: [128, 2(chunk), N]
            # u-side tiles carry both lanes [128, 2(sin|cos), FU] (all
            # DVE-written, so no cross-engine false deps); v-side seed
            # and j3 tiles are split per lane (ACT lane writes / early
            # cos-lane consumers), cv5 is one both-lane tile.
            cu = {j: feat.tile([128, 2, FU], f16, tag=f"cu{j}", name=f"cu{j}")
                  for j in JS}
            us = {j: feat.tile([128, 2, FU], f16, tag=f"us{j}", name=f"us{j}")
                  for j in (1,)}
            cvs = {j: feat.tile([128, 2, N], f16, tag=f"cvs{j}", name=f"cvs{j}") for j in (1, 3)}
            cvc = {j: feat.tile([128, 2, N], f16, tag=f"cvc{j}", name=f"cvc{j}") for j in (1, 3)}

            def useed(lane):
                if bg_zero:
                    bias = hpi[:] if lane else 0.0
                    nc.scalar.activation(cu[1][:, lane, :], thu[:], AF.Sin,
                                         bias=bias)
                else:
                    bl = b1c_sb if lane else b1s_sb
                    for c in range(2):
                        nc.scalar.activation(
                            cu[1][:, lane, c * NH:(c + 1) * NH],
                            thu[:, c * NH:(c + 1) * NH], AF.Sin,
                            bias=bl[:, c:c + 1])

            def vseed(lane, tile_, c):
                if bg_zero:
                    bias = hpi[:] if lane else 0.0
                else:
                    bias = (b1c_sb if lane else b1s_sb)[:, c:c + 1]
                nc.scalar.activation(tile_[:, c, :], thv[c][:], AF.Sin,
                                     bias=bias)

            useed(1)
            useed(0)
            vseed(1, cvc[1], 0)
            vseed(1, cvc[1], 1)
            vseed(0, cvs[1], 0)
            vseed(0, cvs[1], 1)


            def uscale(j):
                for c in range(2):
                    nc.vector.tensor_scalar(
                        us[j][:, :, c * NH:(c + 1) * NH],
                        cu[j][:, :, c * NH:(c + 1) * NH],
                        wav_sb[:, c:c + 1], float(BJ[j]), MULT, MULT)

            sc = [scps.tile([128, NH], f32, tag=f"sc{mb}", name=f"sc{mb}")
                  for mb in range(4)]

            def score_mms(j, rhs_t, first=False):
                lhs = {0: cvc[j], 1: cvs[j]}  # fn=0 pairs us-sin x cv-cos
                for fn in range(2):
                    for c in range(2):
                        for mb in range(4):
                            nc.tensor.matmul(
                                sc[mb][:],
                                lhs[fn][:, c, mb * 128:(mb + 1) * 128],
                                rhs_t[:, fn, c * NH:(c + 1) * NH],
                                start=(first and fn == 0 and c == 0),
                                stop=False,
                                skip_group_check=True,
                            )

            # ---- DVE program (emission order = engine order) ----
            squ = feat.tile([128, FU], f16, tag="squ")
            t2u = feat.tile([128, FU], f16, tag="t2u")
            m3u = feat.tile([128, 2, FU], f16, tag="m3u")
            nc.gpsimd.tensor_copy(biasf[:], xkb_sb[:, 4 * D:])
            nc.vector.tensor_mul(squ[:], cu[1][:, 1, :], cu[1][:, 1, :])
            nc.vector.tensor_scalar(t2u[:], squ[:], 4.0, -2.0, MULT, ADD)
            uscale(1)
            nc.vector.tensor_scalar(m3u[:, 0, :], squ[:], 4.0, -1.0,
                                    MULT, ADD)
            nc.vector.tensor_scalar(m3u[:, 1, :], squ[:], 4.0, -3.0,
                                    MULT, ADD)
            nc.vector.tensor_mul(cu[3][:], cu[1][:], m3u[:])
            # u5 next so ACT can scale us3/us5 in its post-seed window
            t2u_b = t2u[:, None, :].broadcast_to((128, 2, FU))
            tu = feat.tile([128, 2, FU], f16, tag="tu")
            nc.vector.tensor_mul(tu[:], cu[3][:], t2u_b)
            nc.vector.tensor_sub(cu[5][:], tu[:], cu[1][:])
            us3t = feat.tile([128, 2, FU], f16, tag="us3t")
            us5t = feat.tile([128, 2, FU], f16, tag="us5t")
            for c in range(2):
                nc.scalar.activation(
                    us3t[:, :, c * NH:(c + 1) * NH],
                    cu[3][:, :, c * NH:(c + 1) * NH],
                    AF.Identity, scale=wab3_sb[:, c:c + 1])
            for c in range(2):
                nc.scalar.activation(
                    us5t[:, :, c * NH:(c + 1) * NH],
                    cu[5][:, :, c * NH:(c + 1) * NH],
                    AF.Identity, scale=wab5_sb[:, c:c + 1])
            sqv = feat.tile([128, 2, N], f16, tag="sqv")
            m3vp = feat.tile([128, 2, N], f16, tag="m3vp")
            m3vm = feat.tile([128, 2, N], f16, tag="m3vm")
            t2v = feat.tile([128, 2, N], f16, tag="t2v")
            nc.vector.tensor_mul(sqv[:], cvc[1][:], cvc[1][:])
            nc.vector.tensor_scalar(t2v[:], sqv[:], 4.0, -2.0, MULT, ADD)
            nc.vector.tensor_scalar(m3vm[:], sqv[:], 4.0, -3.0, MULT, ADD)
            nc.vector.tensor_mul(cvc[3][:], cvc[1][:], m3vm[:])
            nc.vector.tensor_scalar(m3vp[:], sqv[:], 4.0, -1.0, MULT, ADD)
            nc.vector.tensor_mul(cvs[3][:], cvs[1][:], m3vp[:])
            # ---- PE score program ----
            # filler ladder: each rung anchored on successively later
            # data so the scheduler cannot hoist them; keeps HAM warm
            # from first warmup MM through the j1 score MMs.  The j1
            # start=True MMs re-clear the banks afterwards.
            for f in range(4):
                nc.tensor.matmul(sc[1][:96, :], xkT_sb[:, f, :],
                                 ones[:, :NH], skip_group_check=True)
            for f in range(2):
                nc.tensor.matmul(sc[2][:], cu[1][:, 1, :128],
                                 ones[:, :NH], skip_group_check=True)
            for f in range(2):
                nc.tensor.matmul(sc[3][:], us[1][:, 0, :128],
                                 ones[:, :NH], skip_group_check=True)
            score_mms(1, us[1], first=True)
            score_mms(3, us3t)

            # j5 v-chain emitted per key-half, interleaved with the j5
            # score MMs + sigmoid + out MMs of the matching mb pair
            tv = feat.tile([128, 2, 2, N], f16, tag="tv")
            cv5 = feat.tile([128, 2, 2, N], f16, tag="cv5")
            attT = [consts.tile([128, NH], f16, tag=f"attT{mb}", name=f"attT{mb}")
                    for mb in range(4)]
            t2v_b = t2v[:, None, :, :].broadcast_to((128, 2, 2, N))
            cv1l = {0: cvs[1], 1: cvc[1]}
            cv3l = {0: cvs[3], 1: cvc[3]}
            for h in range(2):
                sl = slice(h * 256, (h + 1) * 256)
                for ln in (1, 0):
                    nc.vector.tensor_mul(
                        tv[:, ln, :, sl], cv3l[ln][:, :, sl],
                        t2v_b[:, ln, :, sl])
                    nc.vector.tensor_sub(
                        cv5[:, ln, :, sl], tv[:, ln, :, sl],
                        cv1l[ln][:, :, sl])
                for mb in (2 * h, 2 * h + 1):
                    for fn in range(2):
                        for c in range(2):
                            nc.tensor.matmul(
                                sc[mb][:],
                                cv5[:, 1 - fn, c, mb * 128:(mb + 1) * 128],
                                us5t[:, fn, c * NH:(c + 1) * NH],
                                start=False,
                                stop=(fn == 1 and c == 1),
                                skip_group_check=True,
                            )
                    nc.scalar.activation(
                        attT[mb][:], sc[mb][:], AF.Tanh, scale=0.5,
                        bias=sgb_sb[:, 0:1]
                    )
                    nc.tensor.matmul(
                        foT[:], xkT_sb[:, mb, :], attT[mb][:],
                        start=False, stop=(mb == 3),
                        skip_group_check=True,
                    )

            out_sb = opool.tile([D, NH], f16, tag="out")
            nc.vector.tensor_copy(out_sb[:, :NH // 2], foT[:, :NH // 2])
            nc.sync.dma_start(out.ap()[:, :NH // 2], out_sb[:, :NH // 2])
            nc.scalar.copy(out_sb[:, NH // 2:], foT[:, NH // 2:])
            nc.sync.dma_start(out.ap()[:, NH // 2:], out_sb[:, NH // 2:])

    nc.compile()
    return nc


def _prep_inputs_v7(x, Wg1, Wg2, bg, Wa_w, Wa_b, ba, bg_zero):
    """Host-side packing/slicing only (no reference math)."""
    x = np.asarray(x, np.float32)
    w1s = (FS * np.asarray(Wg1, np.float32).T).astype(np.float16)
    w2s = (FS * np.asarray(Wg2, np.float32).T).astype(np.float16)
    wac = np.asarray(Wa_w, np.float32).reshape(2, 128).T
    NBC = 7 if bg_zero else 11
    biasc = np.empty((128, NBC), np.float16)
    biasc[:, 0:2] = wac.astype(np.float16)
    biasc[:, 2] = np.float16(0.5 * (float(np.asarray(Wa_b).ravel()[0])
                                    + float(np.asarray(ba).ravel()[0])))
    biasc[:, 3:5] = (wac * np.float32(BJ[3])).astype(np.float16)
    biasc[:, 5:7] = (wac * np.float32(BJ[5])).astype(np.float16)
    if not bg_zero:
        bgv = FS * np.asarray(bg, np.float32)
        biasc[:, 7:9] = bgv.reshape(2, 128).T.astype(np.float16)
        biasc[:, 9:11] = (bgv.reshape(2, 128).T
                          + np.float32(np.pi / 2)).astype(np.float16)
    in_maps = []
    for c in range(NCORES):
        b, half = c // 2, c % 2
        xb = x[b]
        # per-core key permutation: own-half keys first, so xq is a
        # fixed-offset view of xk in every core's (identical) program
        xp = np.concatenate(
            [xb[:, half * NH:(half + 1) * NH],
             xb[:, (1 - half) * NH:(2 - half) * NH]], axis=1)
        vin = np.ascontiguousarray(
            np.concatenate([w1s, xp.astype(np.float16), w2s], axis=1))
        xkTP = ((0.5 * xp.T).astype(np.float16).reshape(4, 128, D)
                .transpose(1, 0, 2).reshape(128, 4 * D))
        xkb = np.ascontiguousarray(np.concatenate([xkTP, biasc], axis=1))
        in_maps.append({"vin": vin, "xkb": xkb})
    return in_maps


def _run(inputs, trace=False):
    from concourse.bass_utils import run_bass_kernel_spmd

    bg_zero = bool(np.all(np.asarray(inputs["bg"]) == 0))
    key = ("nc7b", bg_zero)
    if key not in _cache:
        _cache[key] = _build_nc_v7(bg_zero=bg_zero)
    nc = _cache[key]
    in_maps = _prep_inputs_v7(**inputs, bg_zero=bg_zero)
    res = run_bass_kernel_spmd(
        nc, in_maps, core_ids=list(range(NCORES)), trace=trace
    )
    out = np.empty((B, N, D), np.float32)
    for c in range(NCORES):
        b, half = c // 2, c % 2
        out[b, half * NH:(half + 1) * NH] = \
            res.results[c]["out"].astype(np.float32).T
    return out, res


def kernel(**inputs):
    out, _ = _run(inputs, trace=False)
    return out
